# revision 43
# baseline (speedup 1.0000x reference)
"""GQA kernel for Trainium2, 8 NeuronCores (v2: bf16 + engine-balanced).

Problem: B=4, S=1024, D=2048, 32 q-heads, 8 kv-heads, head_dim=64, fp32.

Sharding: TP-2 over heads x DP-4 over batch. Core c handles batch c//2 and
(for tp = c%2) q-heads [16*tp, 16*tp+16) / kv-heads [4*tp, 4*tp+4). Each core
produces a partial output [1024, 2048] (its heads' contribution to ctx @ Wo);
host sums the two partials per batch. bo is added by the tp=0 core only.

v2 design (cost-model driven; 400us -> 270us):
 - All matmul operands bf16 (host casts x/weights): halves DMA bytes; PE
   matmul rate is the same 1 elem/cycle but transposes drop 1.5->1.0 cyc/row.
 - Phase A: transposes/V-proj per token tile, K-proj per half, Q-proj for
   pairs 0-1 only; DMA issue order tuned so data arrives just before use.
 - Phase B (16 iterations of (pair, half)): scores -> exp -> PV with a
   5-deep... [4-deep] psum rotation for score tiles ([128,512] per head-half),
   exp split per block: 6 on ACT, blocks {2,5} via DVE(scale)->Pool(pow);
   PV accumulation is emission-reordered (middle blocks commute) so the
   2-hop exp chain never stalls PE. Q-proj for pairs 2-7 is interleaved as
   PE filler (jobs 0-7 whole, 8-11 spread over iterations 8-13); iterations
   14-15 carry early O-proj partial accumulations instead.
 - Normalization: DVE copies pv out of psum (frees the single-buffered pv
   banks), DVE reciprocal, Pool partition_broadcast, DVE mul -> bf16 ctxT.
 - Phase C: O-proj, 8-pair psum accumulation per [128,512] out tile, DVE
   bias-add, streamed DMA out; last store split to shorten the drain tail.
 - A junk-transpose warmup keeps PE busy from ~0.9us so the cost model's
   p-state ramp is at speed when real work dispatches; standalone
   EventSemaphore waits on PE.SEQ reset the ramp anchor (avoid multi-sem
   pool-rotation waits on PE: keep each psum score tile single-consumer).
 - Denominators via ones-column in vaug (free: cost is out-free-size only).
PSUM budget: A: ptp 2 + pvt 2 + pk 2 + pq 2 = 8; B: psc 4 + pv 2 + pq2 2 = 8.
"""

import time

import numpy as np
import ml_dtypes

import concourse.bass as bass
import concourse.mybir as mybir
from concourse import bacc
from concourse.tile import TileContext
from concourse.bass_utils import run_bass_kernel_spmd

F32 = mybir.dt.float32
BF16 = mybir.dt.bfloat16

S = 1024          # sequence length
D = 2048          # d_model
NH = 16           # q heads per core
NKV = 4           # kv heads per core
HD = 64           # head dim
QF = NH * HD      # 1024 q features per core
KF = NKV * HD     # 256 kv features per core
KC = D // 128     # 16 contraction chunks of d_model
TT = S // 128     # 8 token tiles
TH = S // 512     # 2 token halves
SCALE = 1.0 / 8.0  # 1/sqrt(64)

N_WARM = 22       # junk warmup transposes
OFF_BLOCKS = (2, 5)   # exp blocks per th-stream routed DVE(scale)->Pool(pow)
# PV emission schedule: at scores-cycle c emit PV of block PV_AT_CYCLE[c].
# Off blocks are delayed >=3 cycles past their scores so the 2-hop exp chain
# never stalls PE; middle blocks commute (only start/stop are order-pinned).
PV_AT_CYCLE = {1: (0,), 2: (1,), 3: (3,), 4: (4,), 5: (2,), 7: (6, 5)}
PV_TAIL = (7,)
LATE_OFF_IT = 99  # from this iteration on, also offload blk 6 hi-half

# pair p -> (lo head, hi head) local q-head indices; lo heads have kv parity 0,
# hi heads kv parity 1 (kv = h // 4; kv 0,2 -> rows 0:64 of kT group kv//2).
LO = [0, 1, 2, 3, 8, 9, 10, 11]
HI = [4, 5, 6, 7, 12, 13, 14, 15]
HEAD_PERM = []
for _p in range(8):
    HEAD_PERM.extend([LO[_p], HI[_p]])

_CACHE = {}
LAST_RUN_NS = None


def _build():
    if "nc" in _CACHE:
        return _CACHE["nc"]

    nc = bacc.Bacc("TRN2", target_bir_lowering=False, debug=False)

    x = nc.dram_tensor("x", [S, D], BF16, kind="ExternalInput").ap()
    wq = nc.dram_tensor("wq", [D, QF], BF16, kind="ExternalInput").ap()
    wk = nc.dram_tensor("wk", [D, KF], BF16, kind="ExternalInput").ap()
    wv = nc.dram_tensor("wv", [D, KF], BF16, kind="ExternalInput").ap()
    wo = nc.dram_tensor("wo", [QF, D], BF16, kind="ExternalInput").ap()
    bq = nc.dram_tensor("bq", [128, 8], F32, kind="ExternalInput").ap()
    bk = nc.dram_tensor("bk", [128, 2], F32, kind="ExternalInput").ap()
    bvb = nc.dram_tensor("bvb", [128, KF], F32, kind="ExternalInput").ap()
    bob = nc.dram_tensor("bob", [128, D], F32, kind="ExternalInput").ap()
    iden = nc.dram_tensor("iden", [128, 128], BF16, kind="ExternalInput").ap()
    out = nc.dram_tensor("out", [S, D], F32, kind="ExternalOutput").ap()

    with TileContext(nc) as tc:
        with (
            tc.tile_pool(name="const", bufs=1) as constp,
            tc.tile_pool(name="persist", bufs=1) as pers,
            tc.tile_pool(name="xsb", bufs=4) as xsbp,
            tc.tile_pool(name="wqp", bufs=2) as wqp,
            tc.tile_pool(name="scp", bufs=4) as scp,
            tc.tile_pool(name="ep", bufs=8) as ep,
            tc.tile_pool(name="np", bufs=2) as npool,
            tc.tile_pool(name="osb", bufs=3) as osbp,
        ):
            # ---- consts / persistent tensors ----
            junk = constp.tile([128, 128], BF16, tag="junk")
            nc.gpsimd.memset(junk[:], 0.0)
            base = constp.tile([128, 1024], BF16, tag="base")
            nc.gpsimd.memset(base[:], float(np.e))

            tid = constp.tile([128, 128], BF16, tag="tid")
            bq_sb = constp.tile([128, 8], F32, tag="bq")
            bk_sb = constp.tile([128, 2], F32, tag="bk")
            bv_sb = constp.tile([128, KF], F32, tag="bv")
            bo_sb = constp.tile([128, D], F32, tag="bo")

            xT = pers.tile([128, KC * S], BF16, tag="xT")  # chunk-major
            kT = [pers.tile([128, S], BF16, tag=f"kT{g}", name=f"kT{g}")
                  for g in range(2)]
            vaug = [pers.tile([128, 65 * TT], BF16, tag=f"va{j}", name=f"va{j}")
                    for j in range(NKV)]
            qT = [pers.tile([128, S], BF16, tag=f"qT{p}", name=f"qT{p}")
                  for p in range(8)]
            ctxT = [pers.tile([128, S], BF16, tag=f"ctxT{p}", name=f"ctxT{p}")
                    for p in range(8)]
            wk_sb = pers.tile([128, KC * KF], BF16, tag="wk")
            wv_sb = pers.tile([128, KC * KF], BF16, tag="wv")
            wo_sb = [pers.tile([128, 8 * 512], BF16, tag=f"wo{nf}", name=f"wo{nf}")
                     for nf in range(4)]

            # vaug ones columns (for softmax denominators via PV matmul)
            for j in range(NKV):
                nc.gpsimd.memset(
                    vaug[j][:].rearrange("p (t c) -> p t c", c=65)[:, :, 64:65],
                    1.0,
                )

            # ---- DMA issue order (DMA engine serializes in issue order) ----
            x_sb = [None] * TT

            def dma_x(t):
                x_sb[t] = xsbp.tile([128, D], BF16, tag="xsb", name=f"xsb{t}")
                nc.sync.dma_start(out=x_sb[t][:], in_=x[128 * t:128 * (t + 1), :])

            dma_x(0)
            nc.sync.dma_start(out=tid[:], in_=iden[:, :])
            for t in range(1, 3):
                dma_x(t)
            nc.sync.dma_start(
                out=wv_sb[:].rearrange("p (c f) -> p c f", c=KC),
                in_=wv.rearrange("(c p) f -> p c f", p=128),
            )
            dma_x(3)
            nc.sync.dma_start(
                out=wk_sb[:].rearrange("p (c f) -> p c f", c=KC),
                in_=wk.rearrange("(c p) f -> p c f", p=128),
            )
            nc.sync.dma_start(out=bq_sb[:], in_=bq[:, :])
            nc.sync.dma_start(out=bk_sb[:], in_=bk[:, :])
            nc.sync.dma_start(out=bv_sb[:], in_=bvb[:, :])

            wq_sb = [None] * 8

            def dma_wq(p):
                wq_sb[p] = wqp.tile([128, KC * 128], BF16, tag="wq", name=f"wq{p}")
                nc.sync.dma_start(
                    out=wq_sb[p][:].rearrange("p (c f) -> p c f", c=KC),
                    in_=wq[:, 128 * p:128 * (p + 1)].rearrange(
                        "(c p) f -> p c f", p=128),
                )

            dma_wq(0)
            dma_wq(1)
            for t in range(4, TT):
                dma_x(t)

            # ---- Phase A: transposes + K/V/Q projections ----
            xTr = xT[:].rearrange("p (c s) -> p c s", s=S)

            with (
                tc.tile_pool(name="ps_tp", bufs=2, space="PSUM") as ps_tp,
                tc.tile_pool(name="ps_v", bufs=2, space="PSUM") as ps_v,
                tc.tile_pool(name="ps_k", bufs=1, space="PSUM") as ps_k,
                tc.tile_pool(name="ps_q", bufs=2, space="PSUM") as ps_q,
            ):
                # warmup: junk transposes to hold the PE p-state ramp
                for i in range(N_WARM):
                    pw = ps_tp.tile([128, 1024], BF16, tag="ptp", name=f"warm{i // 8}")
                    nc.tensor.transpose(
                        pw[:, 128 * (i % 8):128 * (i % 8 + 1)], junk[:], junk[:])

                def do_T(t):
                    # transpose x tile t -> xT[:, c, 128t:128(t+1)] for all c
                    for g in range(2):
                        ptp = ps_tp.tile([128, 1024], BF16, tag="ptp", name=f"ptp{t}_{g}")
                        for i in range(8):
                            c = 8 * g + i
                            nc.tensor.transpose(
                                ptp[:, 128 * i:128 * (i + 1)],
                                x_sb[t][:, 128 * c:128 * (c + 1)],
                                tid[:])
                        nc.scalar.copy(
                            xTr[:, 8 * g:8 * (g + 1), 128 * t:128 * (t + 1)],
                            ptp[:].rearrange("p (i s) -> p i s", s=128),
                        )

                def do_V(t):
                    pvt = ps_v.tile([128, KF], F32, tag="pvt", name=f"pvt{t}")
                    for c in range(KC):
                        nc.tensor.matmul(
                            pvt[:],
                            xTr[:, c, 128 * t:128 * (t + 1)],
                            wv_sb[:, KF * c:KF * (c + 1)],
                            start=(c == 0), stop=(c == KC - 1),
                        )
                    for j in range(NKV):
                        nc.vector.tensor_add(
                            vaug[j][:, 65 * t:65 * t + 64],
                            pvt[:, 64 * j:64 * (j + 1)],
                            bv_sb[:, 64 * j:64 * (j + 1)],
                        )

                def do_K(th):
                    pk = [ps_k.tile([128, 512], F32, tag=f"pk{g}", name=f"pk{g}_{th}")
                          for g in range(2)]
                    for c in range(KC):
                        for g in range(2):
                            nc.tensor.matmul(
                                pk[g][:],
                                wk_sb[:, KF * c + 128 * g:KF * c + 128 * (g + 1)],
                                xT[:, 1024 * c + 512 * th:1024 * c + 512 * (th + 1)],
                                start=(c == 0), stop=(c == KC - 1),
                            )
                    for g in range(2):
                        nc.vector.tensor_scalar_add(
                            kT[g][:, 512 * th:512 * (th + 1)], pk[g][:],
                            bk_sb[:, g:g + 1],
                        )

                def do_Q(p, th):
                    pq = ps_q.tile([128, 512], F32, tag="pq", name=f"pq{p}_{th}")
                    for c in range(KC):
                        nc.tensor.matmul(
                            pq[:],
                            wq_sb[p][:, 128 * c:128 * (c + 1)],
                            xT[:, 1024 * c + 512 * th:1024 * c + 512 * (th + 1)],
                            start=(c == 0), stop=(c == KC - 1),
                        )
                    nc.vector.tensor_scalar_add(
                        qT[p][:, 512 * th:512 * (th + 1)], pq[:],
                        bq_sb[:, p:p + 1],
                    )

                do_T(0)
                do_T(1)
                do_T(2)
                do_V(0)
                do_T(3)
                do_V(1)
                do_V(2)
                do_K(0)
                do_V(3)
                for t in range(4, TT):
                    do_T(t)
                    do_V(t)
                do_K(1)
                for p in range(2):
                    do_Q(p, 0)
                    do_Q(p, 1)
                dma_wq(2)
                dma_wq(3)

                # prefetch wo + bo for phase C
                for nf in range(4):
                    nc.sync.dma_start(
                        out=wo_sb[nf][:].rearrange("q (p f) -> q p f", p=8),
                        in_=wo[:, 512 * nf:512 * (nf + 1)].rearrange(
                            "(p q) f -> q p f", q=128),
                    )
                nc.sync.dma_start(out=bo_sb[:], in_=bob[:, :])

            # ---- Phase B: attention; lazy Q-proj for pairs 2-7 as PE filler ----
            with (
                tc.tile_pool(name="ps_sc", bufs=4, space="PSUM") as ps_sc,
                tc.tile_pool(name="ps_pv", bufs=1, space="PSUM") as ps_pv,
                tc.tile_pool(name="ps_q2", bufs=2, space="PSUM") as ps_q2,
            ):
                def do_lazy_q_chunk(job, c):
                    qp, qth, pq = job
                    nc.tensor.matmul(
                        pq[:],
                        wq_sb[qp][:, 128 * c:128 * (c + 1)],
                        xT[:, 1024 * c + 512 * qth:1024 * c + 512 * (qth + 1)],
                        start=(c == 0), stop=(c == KC - 1),
                    )

                def finish_lazy_q(job):
                    qp, qth, pq = job
                    nc.vector.tensor_scalar_add(
                        qT[qp][:, 512 * qth:512 * (qth + 1)], pq[:],
                        bq_sb[:, qp:qp + 1],
                    )

                # iteration -> [(job index, c0, c1), ...]; jobs 0-7 run whole
                # in one iteration, jobs 8-11 spread over iterations 8-13.
                IT_QWORK = {i: [(i, 0, 16)] for i in range(8)}
                IT_QWORK.update({
                    8: [(8, 0, 11)], 9: [(8, 11, 16), (9, 0, 6)],
                    10: [(9, 6, 16)],
                    11: [(10, 0, 11)], 12: [(10, 11, 16), (11, 0, 6)],
                    13: [(11, 6, 16)],
                })
                jobs = [(qp, qth) for qp in range(2, 8) for qth in range(TH)]
                live_jobs = {}
                po_early = [None, None]
                for it, (p, th) in enumerate(
                        [(p, th) for p in range(8) for th in range(TH)]):
                    glo, ghi = LO[p] // 4 // 2, HI[p] // 4 // 2
                    kvlo, kvhi = LO[p] // 4, HI[p] // 4
                    pvA = ps_pv.tile([65, 512], F32, tag="pvA", name=f"pvA{p}_{th}")
                    pvB = ps_pv.tile([65, 512], F32, tag="pvB", name=f"pvB{p}_{th}")
                    es = [None] * TT
                    # flat list of (job, chunk, is_last) Q-work for this iter
                    qflat = []
                    for ji, c0, c1 in IT_QWORK.get(it, []):
                        if ji not in live_jobs:
                            qp, qth = jobs[ji]
                            pq = ps_q2.tile([128, 512], F32, tag="pq",
                                            name=f"pq{qp}_{qth}")
                            live_jobs[ji] = (qp, qth, pq)
                            if qth == 0 and qp + 2 < 8:
                                dma_wq(qp + 2)
                        for c in range(c0, c1):
                            qflat.append((live_jobs[ji], c, c == 15))
                    if it == 14 or it == 15:
                        # fill bare iterations with early O-proj partials
                        # (nf=0, t=it-14) accumulating pairs 0..6
                        t = it - 14
                        po_early[t] = ps_q2.tile([128, 512], F32, tag="pq",
                                                 name=f"poE{t}")
                        for pp in range(7):
                            nc.tensor.matmul(
                                po_early[t][:],
                                ctxT[pp][:, 128 * t:128 * (t + 1)],
                                wo_sb[0][:, 512 * pp:512 * (pp + 1)],
                                start=(pp == 0), stop=False,
                                skip_group_check=True,
                            )

                    def do_scores_exp(blk):
                        pscL = ps_sc.tile([128, 512], F32, tag="psc")
                        pscH = ps_sc.tile([128, 512], F32, tag="psc")
                        nc.tensor.matmul(
                            pscL[:],
                            kT[glo][0:64, 128 * blk:128 * (blk + 1)],
                            qT[p][0:64, 512 * th:512 * (th + 1)],
                            start=True, stop=True,
                        )
                        nc.tensor.matmul(
                            pscH[:],
                            kT[ghi][64:128, 128 * blk:128 * (blk + 1)],
                            qT[p][64:128, 512 * th:512 * (th + 1)],
                            start=True, stop=True,
                        )
                        e = ep.tile([128, 1024], BF16, tag="e")
                        for half, psch in ((0, pscL), (1, pscH)):
                            off = blk in OFF_BLOCKS or (
                                it >= LATE_OFF_IT and blk == 6 and half == 1)
                            if off:
                                sc = scp.tile([128, 512], BF16, tag="sc")
                                nc.vector.tensor_scalar(
                                    sc[:], psch[:], SCALE, None,
                                    op0=mybir.AluOpType.mult,
                                )
                                nc.gpsimd.tensor_tensor(
                                    e[:, 512 * half:512 * (half + 1)],
                                    base[:, 0:512], sc[:], mybir.AluOpType.pow)
                            else:
                                nc.scalar.activation(
                                    e[:, 512 * half:512 * (half + 1)], psch[:],
                                    mybir.ActivationFunctionType.Exp,
                                    bias=0.0, scale=SCALE,
                                )
                        es[blk] = e

                    def do_pv(pb):
                        nc.tensor.matmul(
                            pvA[:],
                            vaug[kvlo][:, 65 * pb:65 * pb + 65],
                            es[pb][:, 0:512],
                            start=(pb == 0), stop=(pb == TT - 1),
                        )
                        nc.tensor.matmul(
                            pvB[:],
                            vaug[kvhi][:, 65 * pb:65 * pb + 65],
                            es[pb][:, 512:1024],
                            start=(pb == 0), stop=(pb == TT - 1),
                        )

                    nsteps = len(qflat)
                    for blk in range(TT):
                        do_scores_exp(blk)
                        for qjob, c, is_last in qflat[blk * nsteps // TT:
                                                     (blk + 1) * nsteps // TT]:
                            do_lazy_q_chunk(qjob, c)
                            if is_last:
                                finish_lazy_q(qjob)
                        for pb in PV_AT_CYCLE.get(blk, ()):
                            do_pv(pb)
                    for pb in PV_TAIL:
                        do_pv(pb)

                    # normalize: copy pv to SBUF first (frees psum), then
                    # reciprocal (DVE) + broadcast/mul (Pool, all-SBUF).
                    cpA = npool.tile([65, 512], F32, tag="cpA")
                    cpB = npool.tile([65, 512], F32, tag="cpB")
                    nc.vector.tensor_copy(cpA[:], pvA[:])
                    nc.vector.tensor_copy(cpB[:], pvB[:])
                    recA = npool.tile([1, 512], F32, tag="recA")
                    recB = npool.tile([1, 512], F32, tag="recB")
                    nc.vector.reciprocal(recA[:], cpA[64:65, :])
                    nc.vector.reciprocal(recB[:], cpB[64:65, :])
                    bcA = npool.tile([64, 512], F32, tag="bcA")
                    bcB = npool.tile([64, 512], F32, tag="bcB")
                    nc.gpsimd.partition_broadcast(bcA[:], recA[:])
                    nc.gpsimd.partition_broadcast(bcB[:], recB[:])
                    nc.vector.tensor_mul(
                        ctxT[p][0:64, 512 * th:512 * (th + 1)],
                        cpA[0:64, :], bcA[:],
                    )
                    nc.vector.tensor_mul(
                        ctxT[p][64:128, 512 * th:512 * (th + 1)],
                        cpB[0:64, :], bcB[:],
                    )

                # finish the early O-proj partials (add pair 7, bias, store)
                for t in range(2):
                    nc.tensor.matmul(
                        po_early[t][:],
                        ctxT[7][:, 128 * t:128 * (t + 1)],
                        wo_sb[0][:, 512 * 7:512 * 8],
                        start=False, stop=True,
                        skip_group_check=True,
                    )
                    o_sb = osbp.tile([128, 512], F32, tag="osb", name=f"osbE{t}")
                    nc.vector.tensor_add(
                        o_sb[:], po_early[t][:], bo_sb[:, 0:512])
                    nc.sync.dma_start(
                        out=out[128 * t:128 * (t + 1), 0:512],
                        in_=o_sb[:],
                    )

            # ---- Phase C: output projection ----
            with tc.tile_pool(name="ps_o", bufs=4, space="PSUM") as ps_o:
                for nf in range(4):
                    for t in range(TT):
                        if nf == 0 and t < 2:
                            continue  # done as early partials in phase B
                        po = ps_o.tile([128, 512], F32, tag="po", name=f"po{nf}_{t}")
                        for p in range(8):
                            nc.tensor.matmul(
                                po[:],
                                ctxT[p][:, 128 * t:128 * (t + 1)],
                                wo_sb[nf][:, 512 * p:512 * (p + 1)],
                                start=(p == 0), stop=(p == 7),
                            )
                        o_sb = osbp.tile([128, 512], F32, tag="osb",
                                         name=f"osb{nf}_{t}")
                        if nf == 3 and t == TT - 1:
                            # split the final store so the exposed tail after
                            # the last matmul is half an add + half a DMA
                            for h in range(2):
                                nc.vector.tensor_add(
                                    o_sb[:, 256 * h:256 * (h + 1)],
                                    po[:, 256 * h:256 * (h + 1)],
                                    bo_sb[:, 512 * nf + 256 * h:
                                          512 * nf + 256 * (h + 1)])
                                nc.sync.dma_start(
                                    out=out[128 * t:128 * (t + 1),
                                            512 * nf + 256 * h:
                                            512 * nf + 256 * (h + 1)],
                                    in_=o_sb[:, 256 * h:256 * (h + 1)],
                                )
                        else:
                            nc.vector.tensor_add(
                                o_sb[:], po[:], bo_sb[:, 512 * nf:512 * (nf + 1)])
                            nc.sync.dma_start(
                                out=out[128 * t:128 * (t + 1),
                                        512 * nf:512 * (nf + 1)],
                                in_=o_sb[:],
                            )

    nc.compile()
    _CACHE["nc"] = nc
    return nc


def _prep_core_inputs(c, x, Wq, bq, Wk, bk, Wv, bv, Wo, bo):
    bf16 = ml_dtypes.bfloat16
    tp = c % 2
    b = c // 2
    hperm = [16 * tp + h for h in HEAD_PERM]

    wq_c = np.ascontiguousarray(
        Wq.reshape(D, 32, HD)[:, hperm, :].reshape(D, QF)).astype(bf16)
    bq_c = np.ascontiguousarray(
        bq.reshape(32, HD)[hperm].reshape(8, 128).T).astype(np.float32)
    wk_c = np.ascontiguousarray(Wk[:, KF * tp:KF * (tp + 1)]).astype(bf16)
    bk_c = np.ascontiguousarray(
        bk[KF * tp:KF * (tp + 1)].reshape(2, 128).T).astype(np.float32)
    wv_c = np.ascontiguousarray(Wv[:, KF * tp:KF * (tp + 1)]).astype(bf16)
    bv_c = bv[KF * tp:KF * (tp + 1)]
    bvb = np.tile(bv_c[None, :], (128, 1)).astype(np.float32)
    wo_c = np.ascontiguousarray(
        Wo.reshape(32, HD, D)[hperm].reshape(QF, D)).astype(bf16)
    if tp == 0:
        bob = np.tile(bo[None, :], (128, 1)).astype(np.float32)
    else:
        bob = np.zeros((128, D), np.float32)
    return {
        "x": np.ascontiguousarray(x[b]).astype(bf16),
        "wq": wq_c, "wk": wk_c, "wv": wv_c, "wo": wo_c,
        "bq": bq_c, "bk": bk_c,
        "bvb": np.ascontiguousarray(bvb),
        "bob": np.ascontiguousarray(bob),
        "iden": np.eye(128, dtype=bf16),
    }


def kernel(x, Wq, bq, Wk, bk, Wv, bv, Wo, bo):
    global LAST_RUN_NS
    nc = _build()
    in_maps = [
        _prep_core_inputs(c, x, Wq, bq, Wk, bk, Wv, bv, Wo, bo) for c in range(8)
    ]
    t0 = time.perf_counter_ns()
    res = run_bass_kernel_spmd(nc, in_maps, list(range(8)))
    LAST_RUN_NS = time.perf_counter_ns() - t0
    parts = [res.results[c]["out"] for c in range(8)]
    out = np.empty((4, S, D), np.float32)
    for b in range(4):
        out[b] = parts[2 * b] + parts[2 * b + 1]
    return out
